# revision 14
# baseline (speedup 1.0000x reference)
"""Bahdanau (concat/additive) attention on 8 Trainium2 NeuronCores.

Reference (per batch b):
  context_p = context @ Wh_w.T + Wh_b          # [S, A]
  output_p  = output @ Ws_w.T                  # [A]
  tmp       = tanh(context_p + output_p)       # [S, A]
  scores    = tmp @ v_w.T                      # [S]
  scores    = where(mask==0, -1e9, scores)
  p         = softmax(scores)                  # [S]
  weighted  = p @ context                      # [V]
  returns (weighted [B,V] f32, p [B,1,S] f32)

Strategy: pure data-parallel over batch (B=32 -> 4 per core), no
collectives. bf16 TensorE compute with f32 PSUM accumulation. The host
pre-packs inputs into the exact SBUF layouts the device wants:
  - ctxT  [BL,8,128,S]  context transposed (v on partitions) for phase A
  - ctx   [BL,S,V]      natural layout for the weighted-sum phase
  - whT/wsT [8,128,A]   weight transposes (contraction dim on partitions)
Phase A per (batch, 512-wide s-block): 8x8 accumulation-group matmuls
produce context_p^T [a,s] in PSUM, ScalarE applies tanh with the
per-partition bias (Wh_b + output_p), PE reduces against v_w into
scores [1,512], and VectorE folds the additive mask while copying
scores out of PSUM. Softmax runs without max-subtraction (|scores| is
small by construction; masked entries are -1e9 so exp underflows to
+0.0 exactly like the reference), p goes back to [128,16] layout via a
4KB DRAM round-trip + xbar DMA transpose, and the weighted sum streams
ctx in natural layout with p as the stationary operand (scaled by
1/sum at the PSUM->SBUF copy).

Startup is latency-tuned: whT and the first half of batch 0's ctxT go
first on the Sync HWDGE ring so the first matmul can issue ~14us in;
the small constants ride the Scalar HWDGE ring; the output_p matmuls
are injected into the PE stream two accumulation groups into batch 0
(before the PSUM pool would force a tanh->comb dependency stall).
"""

import sys

sys.path.insert(0, "/opt/trn_rl_repo")

import ml_dtypes
import numpy as np

import concourse.bass as bass  # noqa: F401
import concourse.mybir as mybir
import concourse.tile as tile
from concourse import bacc
from concourse.bass_utils import run_bass_kernel_spmd

B, S, QD, VD, AD = 32, 2048, 1024, 1024, 1024
N_CORES = 8
BL = B // N_CORES  # batches per core
VC, AC, QC = VD // 128, AD // 128, QD // 128  # 128-partition chunks
SB = 4  # s-blocks per batch
SBW = S // SB  # s-block width (512)
SCH = S // 128  # 128-wide s-chunks (16)
HW = S // 2  # ctxT half width (1024)

F32 = mybir.dt.float32
BF16 = mybir.dt.bfloat16
AF = mybir.ActivationFunctionType
nbf = ml_dtypes.bfloat16


def build_nc():
    nc = bacc.Bacc("TRN2", target_bir_lowering=False, debug=False)

    ctxT_d = nc.dram_tensor("ctxT", [BL, VC, 128, S], BF16, kind="ExternalInput")
    ctx_d = nc.dram_tensor("ctx", [BL, S, VD], BF16, kind="ExternalInput")
    whT_d = nc.dram_tensor("whT", [VC, 128, AD], BF16, kind="ExternalInput")
    wsT_d = nc.dram_tensor("wsT", [QC, 128, AD], BF16, kind="ExternalInput")
    outT_d = nc.dram_tensor("outT", [128, QC, BL], BF16, kind="ExternalInput")
    whb_d = nc.dram_tensor("whb", [128, AC], F32, kind="ExternalInput")
    vre_d = nc.dram_tensor("vre", [128, AC], BF16, kind="ExternalInput")
    mb_d = nc.dram_tensor("mb", [BL, S], BF16, kind="ExternalInput")
    wout_d = nc.dram_tensor("weighted", [BL, VD], F32, kind="ExternalOutput")
    pout_d = nc.dram_tensor("p_attn", [BL, S], F32, kind="ExternalOutput")

    with tile.TileContext(nc) as tc:
        with (
            tc.tile_pool(name="const", bufs=1) as constp,
            tc.tile_pool(name="ctxT", bufs=4 * VC) as ctxTp,
            tc.tile_pool(name="ctxB", bufs=16) as ctxBp,
            tc.tile_pool(name="tmp", bufs=10) as tmpp,
            tc.tile_pool(name="sm", bufs=1) as smp,
            tc.tile_pool(name="mbp", bufs=2) as mbp,
            tc.tile_pool(name="pTp", bufs=2) as pTp,
            tc.tile_pool(name="dramp", bufs=2, space="DRAM") as dramp,
            tc.tile_pool(name="pscp", bufs=4, space="PSUM") as pscp,
            tc.tile_pool(name="pssc", bufs=1, space="PSUM") as psscp,
            tc.tile_pool(name="pswp", bufs=1, space="PSUM") as pswp,
            tc.tile_pool(name="pcmb", bufs=1, space="PSUM") as pcmbp,
        ):
            # ---- PE warmup: dummy matmuls keep the HAM activity monitor
            # busy during the ~10us NEFF/DMA startup ramp so the real
            # matmuls start at 2.4GHz instead of 1.2GHz ----
            warm_sb = constp.tile([128, SBW], BF16, tag="warm")
            nc.vector.memset(warm_sb, 0.0)
            pwarm = pcmbp.tile([128, SBW], F32, tag="cmb", name="pwarm")
            for _ in range(12):
                nc.tensor.matmul(pwarm, warm_sb[:, 0:128], warm_sb, start=True, stop=True)


            ctxT_tiles = {}  # (b, vc, half) -> tile

            def emit_ctxT_dma(b, halves=(0, 1), split=False):
                for h in halves:
                    for vc in range(VC):
                        t = ctxTp.tile(
                            [128, HW], BF16, tag="ctxT", name=f"ctxT{b}_{vc}_{h}"
                        )
                        ctxT_tiles[(b, vc, h)] = t
                    if split:
                        # two region DMAs per tile, all-vc low halves first,
                        # so the first s-block gates on 1MB instead of 2MB
                        for q in (0, 1):
                            for vc in range(VC):
                                t = ctxT_tiles[(b, vc, h)]
                                nc.sync.dma_start(
                                    out=t[:, q * SBW : (q + 1) * SBW],
                                    in_=ctxT_d[
                                        b, vc, :,
                                        h * HW + q * SBW : h * HW + (q + 1) * SBW,
                                    ],
                                )
                    else:
                        for vc in range(VC):
                            t = ctxT_tiles[(b, vc, h)]
                            nc.sync.dma_start(
                                out=t, in_=ctxT_d[b, vc, :, h * HW : (h + 1) * HW]
                            )

            # pairwise whT[vc] / ctxT(0,vc,h0) so the vc-th matmul of the
            # first accumulation group gates on only (vc+1)*0.5MB; both waits
            # are region/tile granular so the group JIT-streams the startup DMA
            whT_sb = constp.tile([128, VC, AD], BF16, tag="whT")
            for vc in range(VC):
                nc.sync.dma_start(out=whT_sb[:, vc, :], in_=whT_d[vc])
                t = ctxTp.tile([128, HW], BF16, tag="ctxT", name=f"ctxT0_{vc}_0")
                nc.sync.dma_start(out=t, in_=ctxT_d[0, vc, :, 0:HW])
                ctxT_tiles[(0, vc, 0)] = t

            # small constants on the Scalar HWDGE ring (off the big stream)
            outT_sb = constp.tile([128, QC, BL], BF16, tag="outT")
            nc.scalar.dma_start(out=outT_sb, in_=outT_d[:, :, :])
            whb_sb = constp.tile([128, AC], F32, tag="whb")
            nc.scalar.dma_start(out=whb_sb, in_=whb_d[:, :])
            vre_sb = constp.tile([128, AC], BF16, tag="vre")
            nc.scalar.dma_start(out=vre_sb, in_=vre_d[:, :])

            wsT_sb = constp.tile([128, QC, AD], BF16, tag="wsT")
            for qc in range(QC):
                nc.sync.dma_start(out=wsT_sb[:, qc, :], in_=wsT_d[qc])
            emit_ctxT_dma(0, (1,))
            emit_ctxT_dma(1)

            comb_sb = constp.tile([128, AC, BL], F32, tag="comb")

            def emit_comb():
                """comb[a,(ac,b)] = Wh_b[a] + (output @ Ws_w.T)[b,a].
                One PSUM tile, 8 accumulation regions -> no pool stalls."""
                pcmb = pcmbp.tile([128, AC * BL], F32, tag="cmb")
                for ac in range(AC):
                    reg = pcmb[:, ac * BL : (ac + 1) * BL]
                    for qc in range(QC):
                        nc.tensor.matmul(
                            reg,
                            wsT_sb[:, qc, ac * 128 : (ac + 1) * 128],
                            outT_sb[:, qc, :],
                            start=(qc == 0),
                            stop=(qc == QC - 1),
                        )
                for ac in range(AC):
                    nc.vector.tensor_scalar_add(
                        comb_sb[:, ac, :],
                        pcmb[:, ac * BL : (ac + 1) * BL],
                        whb_sb[:, ac : ac + 1],
                    )

            scores_rows = {}
            mb_tiles = {}
            pT_tiles = {}
            rs_tiles = {}
            pbf_tiles = {}
            pscr_tiles = {}
            ssum_tiles = {}

            def emit_A(b, sblocks, inject_comb=False):
                if b not in scores_rows:
                    scores_rows[b] = smp.tile(
                        [1, S], F32, tag="scores", name=f"scores{b}", bufs=2
                    )
                    mb_tiles[b] = mbp.tile([1, S], BF16, tag="mb", name=f"mb{b}")
                    nc.scalar.dma_start(out=mb_tiles[b], in_=mb_d[b])
                scores_row = scores_rows[b]
                mb_t = mb_tiles[b]

                def emit_tanh(ac, ps):
                    tm = tmpp.tile([128, SBW], BF16, tag="tm", name=f"tm{b}_{ac}")
                    nc.scalar.activation(
                        out=tm,
                        in_=ps,
                        func=AF.Tanh,
                        bias=comb_sb[:, ac, b : b + 1],
                        scale=1.0,
                    )
                    return tm

                for sb in sblocks:
                    h, ssl = sb // 2, slice((sb % 2) * SBW, (sb % 2) * SBW + SBW)
                    osl = slice(sb * SBW, (sb + 1) * SBW)
                    tmps = []
                    deferred = []
                    for ac in range(AC):
                        ps = pscp.tile([128, SBW], F32, tag="cp")
                        for vc in range(VC):
                            nc.tensor.matmul(
                                ps,
                                whT_sb[:, vc, ac * 128 : (ac + 1) * 128],
                                ctxT_tiles[(b, vc, h)][:, ssl],
                                start=(vc == 0),
                                stop=(vc == VC - 1),
                            )
                        if inject_comb and sb == sblocks[0] and ac < 4:
                            # comb_sb must be emitted before any tanh reads it
                            # (Tile tracks RAW in emission order), but the comb
                            # matmuls must also precede the 3rd accumulation
                            # group in the PE stream (pscp has 3 slots).
                            deferred.append((ac, ps))
                            tmps.append(None)
                            if ac == 3:
                                emit_comb()
                                for ac2, ps2 in deferred:
                                    tmps[ac2] = emit_tanh(ac2, ps2)
                            continue
                        tmps.append(emit_tanh(ac, ps))
                    last_sb = b == BL - 1 and sb == SB - 1
                    if last_sb:
                        # tail-latency-critical: serial accumulation avoids the
                        # 4-partial combine chain on the critical path
                        pssc = psscp.tile([128, SBW], F32, tag="sc")
                        for ac in range(AC):
                            nc.tensor.matmul(
                                pssc[0:1, :],
                                vre_sb[:, ac : ac + 1],
                                tmps[ac],
                                start=(ac == 0),
                                stop=(ac == AC - 1),
                            )
                        nc.vector.tensor_add(
                            scores_row[0:1, osl], pssc[0:1, :], mb_t[0:1, osl]
                        )
                    else:
                        pssc = psscp.tile([128, SBW], F32, tag="sc")
                        for r in range(2):
                            for j in range(4):
                                ac = r * 4 + j
                                nc.tensor.matmul(
                                    pssc[32 * j : 32 * j + 1, :],
                                    vre_sb[:, ac : ac + 1],
                                    tmps[ac],
                                    start=(r == 0),
                                    stop=(r == 1),
                                    tile_position=(0, 32 * j),
                                )
                        # combine the 4 col-group partials; mask folds into op 1
                        t0 = smp.tile([1, SBW], F32, tag="sct0", bufs=1)
                        nc.vector.tensor_add(t0, pssc[0:1, :], mb_t[0:1, osl])
                        t1 = smp.tile([1, SBW], F32, tag="sct1", bufs=1)
                        nc.vector.tensor_add(t1, pssc[32:33, :], t0)
                        t2 = smp.tile([1, SBW], F32, tag="sct2", bufs=1)
                        nc.vector.tensor_add(t2, pssc[64:65, :], t1)
                        nc.vector.tensor_add(scores_row[0:1, osl], pssc[96:97, :], t2)
                    # exp per s-block: spreads ACT work and shortens the
                    # scores->pT critical path after the last s-block
                    if b not in pbf_tiles:
                        pbf_tiles[b] = smp.tile(
                            [1, S], BF16, tag="pbf", name=f"pbf{b}", bufs=2
                        )
                        pscr_tiles[b] = dramp.tile(
                            [1, S], BF16, tag="pscr", name=f"pscr{b}"
                        )
                        ssum_tiles[b] = smp.tile(
                            [1, SB], F32, tag="ssum", name=f"ssum{b}", bufs=2
                        )
                    if last_sb:
                        # chunked exp + pscr so only ~128 elements sit on the
                        # critical path before the xbar transpose
                        ss4 = smp.tile([1, 4], F32, tag="ss4", bufs=1)
                        for q in range(4):
                            qsl = slice(sb * SBW + q * 128, sb * SBW + (q + 1) * 128)
                            nc.scalar.activation(
                                out=pbf_tiles[b][0:1, qsl],
                                in_=scores_row[0:1, qsl],
                                func=AF.Exp,
                                accum_out=ss4[0:1, q : q + 1],
                            )
                            nc.sync.dma_start(
                                out=pscr_tiles[b][0:1, qsl],
                                in_=pbf_tiles[b][0:1, qsl],
                            )
                        nc.vector.reduce_sum(
                            out=ssum_tiles[b][0:1, sb : sb + 1],
                            in_=ss4[0:1, :],
                            axis=mybir.AxisListType.X,
                        )
                    else:
                        nc.scalar.activation(
                            out=pbf_tiles[b][0:1, osl],
                            in_=scores_row[0:1, osl],
                            func=AF.Exp,
                            accum_out=ssum_tiles[b][0:1, sb : sb + 1],
                        )
                        nc.sync.dma_start(
                            out=pscr_tiles[b][0:1, osl], in_=pbf_tiles[b][0:1, osl]
                        )

            def emit_softmax(b):
                pbf = pbf_tiles[b]
                pT = pTp.tile([128, SCH], BF16, tag="pT", name=f"pT{b}")
                nc.sync.dma_start(
                    out=pT,
                    in_=pscr_tiles[b].rearrange("o (r c) -> (o r) c", c=128),
                    transpose=True,
                )
                pT_tiles[b] = pT
                ssum = ssum_tiles[b]
                stot = smp.tile([1, 1], F32, tag="stot", bufs=2)
                nc.vector.reduce_sum(
                    out=stot, in_=ssum[0:1, :], axis=mybir.AxisListType.X
                )
                rs = smp.tile([1, 1], F32, tag="rs", name=f"rs{b}", bufs=2)
                nc.vector.reciprocal(out=rs, in_=stot)
                rs_tiles[b] = rs

            def emit_pout(b):
                pf = smp.tile([1, S], F32, tag="pf")
                nc.vector.tensor_scalar_mul(pf, pbf_tiles[b], rs_tiles[b][0:1, 0:1])
                nc.scalar.dma_start(out=pout_d[b], in_=pf)

            def emit_B(b):
                psw = pswp.tile([128, VD], F32, tag="w")
                pT = pT_tiles[b]
                for sc in range(SCH):
                    j, r = sc % 4, sc // 4
                    cx = ctxBp.tile([128, VD], BF16, tag="cx")
                    nc.gpsimd.dma_start(
                        out=cx, in_=ctx_d[b, sc * 128 : (sc + 1) * 128, :]
                    )
                    for vh in range(VD // SBW):
                        nc.tensor.matmul(
                            psw[32 * j : 32 * j + 1, vh * SBW : (vh + 1) * SBW],
                            pT[:, sc : sc + 1],
                            cx[:, vh * SBW : (vh + 1) * SBW],
                            start=(r == 0),
                            stop=(r == SCH // 4 - 1),
                            tile_position=(0, 32 * j),
                        )
                bt0 = smp.tile([1, VD], F32, tag="bt0")
                nc.vector.tensor_copy(out=bt0, in_=psw[0:1, :])
                bt1 = smp.tile([1, VD], F32, tag="bt1")
                nc.vector.tensor_add(bt1, psw[32:33, :], bt0)
                bt2 = smp.tile([1, VD], F32, tag="bt2")
                nc.vector.tensor_add(bt2, psw[64:65, :], bt1)
                bt3 = smp.tile([1, VD], F32, tag="bt3")
                nc.vector.tensor_add(bt3, psw[96:97, :], bt2)
                wsb = smp.tile([1, VD], F32, tag="wsb")
                # weighted = (exp(s) @ ctx) / sum(exp(s))
                nc.vector.tensor_scalar_mul(wsb, bt3, rs_tiles[b][0:1, 0:1])
                nc.scalar.dma_start(out=wout_d[b], in_=wsb)

            # ---- software-pipelined emission ----
            emit_A(0, [0], inject_comb=True)
            emit_A(0, [1, 2, 3])
            emit_ctxT_dma(2)
            emit_softmax(0)
            emit_A(1, [0])
            emit_B(0)
            emit_pout(0)
            emit_A(1, [1, 2, 3])
            emit_ctxT_dma(3)
            emit_softmax(1)
            emit_A(2, [0])
            emit_B(1)
            emit_pout(1)
            emit_A(2, [1, 2, 3])
            emit_softmax(2)
            emit_A(3, [0])
            emit_B(2)
            emit_pout(2)
            emit_A(3, [1, 2, 3])
            emit_softmax(3)
            emit_B(3)
            emit_pout(3)

    nc.compile()
    return nc


def _host_pack(output, context, attn_mask, Wh_w, Wh_b, Ws_w, v_w):
    """Build per-core in_maps with device-friendly layouts/dtypes."""
    ctx_bf = context.astype(nbf)  # [B, S, V]
    # ctxT[b, vc, p, s] = context[b, s, vc*128+p]
    ctxT_bf = np.ascontiguousarray(
        ctx_bf.transpose(0, 2, 1).reshape(B, VC, 128, S)
    )
    whT = np.ascontiguousarray(Wh_w.T.reshape(VC, 128, AD)).astype(nbf)
    wsT = np.ascontiguousarray(Ws_w.T.reshape(QC, 128, AD)).astype(nbf)
    # outT[p, qc, b] = output[b, qc*128+p]  (per core slice of b)
    outT_all = np.ascontiguousarray(
        output.T.reshape(QC, 128, B).transpose(1, 0, 2)
    ).astype(nbf)
    whb = np.ascontiguousarray(Wh_b.reshape(AC, 128).T).astype(np.float32)
    vre = np.ascontiguousarray(v_w.reshape(AC, 128).T).astype(nbf)
    mb = np.where(attn_mask[:, 0, :] == 0, np.float32(-1e9), np.float32(0.0))
    mb = np.ascontiguousarray(mb).astype(nbf)

    in_maps = []
    for c in range(N_CORES):
        bs = slice(c * BL, (c + 1) * BL)
        in_maps.append(
            {
                "ctxT": np.ascontiguousarray(ctxT_bf[bs]),
                "ctx": np.ascontiguousarray(ctx_bf[bs]),
                "whT": whT,
                "wsT": wsT,
                "outT": np.ascontiguousarray(outT_all[:, :, bs]),
                "whb": whb,
                "vre": vre,
                "mb": np.ascontiguousarray(mb[bs]),
            }
        )
    return in_maps


_NC_CACHE = []


def get_nc():
    if not _NC_CACHE:
        _NC_CACHE.append(build_nc())
    return _NC_CACHE[0]


def kernel(output, context, attn_mask, Wh_w, Wh_b, Ws_w, v_w, _trace=False):
    output = np.asarray(output, dtype=np.float32)
    context = np.asarray(context, dtype=np.float32)
    attn_mask = np.asarray(attn_mask)
    Wh_w = np.asarray(Wh_w, dtype=np.float32)
    Wh_b = np.asarray(Wh_b, dtype=np.float32)
    Ws_w = np.asarray(Ws_w, dtype=np.float32)
    v_w = np.asarray(v_w, dtype=np.float32)

    nc = get_nc()
    in_maps = _host_pack(output, context, attn_mask, Wh_w, Wh_b, Ws_w, v_w)
    kw = {}
    if _trace:
        kw = dict(trace=True)
    res = run_bass_kernel_spmd(nc, in_maps, core_ids=list(range(N_CORES)), **kw)
    weighted = np.concatenate(
        [res.results[c]["weighted"] for c in range(N_CORES)], axis=0
    ).astype(np.float32)
    p_attn = np.concatenate(
        [res.results[c]["p_attn"][:, None, :] for c in range(N_CORES)], axis=0
    ).astype(np.float32)
    if _trace:
        return (weighted, p_attn), res
    return weighted, p_attn


if __name__ == "__main__":
    rng = np.random.default_rng(0)
    inputs = {
        "output": rng.standard_normal((B, QD), dtype=np.float32),
        "context": rng.standard_normal((B, S, VD), dtype=np.float32),
        "attn_mask": rng.integers(0, 2, (B, 1, S)).astype(np.int32),
        "Wh_w": (rng.standard_normal((AD, VD), dtype=np.float32) * 0.02),
        "Wh_b": (rng.standard_normal((AD,), dtype=np.float32) * 0.02),
        "Ws_w": (rng.standard_normal((AD, QD), dtype=np.float32) * 0.02),
        "v_w": (rng.standard_normal((1, AD), dtype=np.float32) * 0.02),
    }
    w, p = kernel(**inputs)
    print("weighted", w.shape, w.dtype, "p_attn", p.shape, p.dtype)


# revision 15
# speedup vs baseline: 1.0036x; 1.0036x over previous
"""Bahdanau (concat/additive) attention on 8 Trainium2 NeuronCores.

Reference (per batch b):
  context_p = context @ Wh_w.T + Wh_b          # [S, A]
  output_p  = output @ Ws_w.T                  # [A]
  tmp       = tanh(context_p + output_p)       # [S, A]
  scores    = tmp @ v_w.T                      # [S]
  scores    = where(mask==0, -1e9, scores)
  p         = softmax(scores)                  # [S]
  weighted  = p @ context                      # [V]
  returns (weighted [B,V] f32, p [B,1,S] f32)

Strategy: pure data-parallel over batch (B=32 -> 4 per core), no
collectives. bf16 TensorE compute with f32 PSUM accumulation. The host
pre-packs inputs into the exact SBUF layouts the device wants:
  - ctxT  [BL,8,128,S]  context transposed (v on partitions) for phase A
  - ctx   [BL,S,V]      natural layout for the weighted-sum phase
  - whT/wsT [8,128,A]   weight transposes (contraction dim on partitions)
Phase A per (batch, 512-wide s-block): 8x8 accumulation-group matmuls
produce context_p^T [a,s] in PSUM, ScalarE applies tanh with the
per-partition bias (Wh_b + output_p), PE reduces against v_w into
scores [1,512], and VectorE folds the additive mask while copying
scores out of PSUM. Softmax runs without max-subtraction (|scores| is
small by construction; masked entries are -1e9 so exp underflows to
+0.0 exactly like the reference), p goes back to [128,16] layout via a
4KB DRAM round-trip + xbar DMA transpose, and the weighted sum streams
ctx in natural layout with p as the stationary operand (scaled by
1/sum at the PSUM->SBUF copy).

Startup is latency-tuned: whT and the first half of batch 0's ctxT go
first on the Sync HWDGE ring so the first matmul can issue ~14us in;
the small constants ride the Scalar HWDGE ring; the output_p matmuls
are injected into the PE stream two accumulation groups into batch 0
(before the PSUM pool would force a tanh->comb dependency stall).
"""

import sys

sys.path.insert(0, "/opt/trn_rl_repo")

import ml_dtypes
import numpy as np

import concourse.bass as bass  # noqa: F401
import concourse.mybir as mybir
import concourse.tile as tile
from concourse import bacc
from concourse.bass_utils import run_bass_kernel_spmd

B, S, QD, VD, AD = 32, 2048, 1024, 1024, 1024
N_CORES = 8
BL = B // N_CORES  # batches per core
VC, AC, QC = VD // 128, AD // 128, QD // 128  # 128-partition chunks
SB = 4  # s-blocks per batch
SBW = S // SB  # s-block width (512)
SCH = S // 128  # 128-wide s-chunks (16)
HW = S // 2  # ctxT half width (1024)

F32 = mybir.dt.float32
BF16 = mybir.dt.bfloat16
AF = mybir.ActivationFunctionType
nbf = ml_dtypes.bfloat16


def build_nc():
    nc = bacc.Bacc("TRN2", target_bir_lowering=False, debug=False)

    ctxT_d = nc.dram_tensor("ctxT", [BL, VC, 128, S], BF16, kind="ExternalInput")
    ctx_d = nc.dram_tensor("ctx", [BL, S, VD], BF16, kind="ExternalInput")
    whT_d = nc.dram_tensor("whT", [VC, 128, AD], BF16, kind="ExternalInput")
    wsT_d = nc.dram_tensor("wsT", [QC, 128, AD], BF16, kind="ExternalInput")
    outT_d = nc.dram_tensor("outT", [128, QC, BL], BF16, kind="ExternalInput")
    whb_d = nc.dram_tensor("whb", [128, AC], F32, kind="ExternalInput")
    vre_d = nc.dram_tensor("vre", [128, AC], BF16, kind="ExternalInput")
    mb_d = nc.dram_tensor("mb", [BL, S], BF16, kind="ExternalInput")
    wout_d = nc.dram_tensor("weighted", [BL, VD], F32, kind="ExternalOutput")
    pout_d = nc.dram_tensor("p_attn", [BL, S], F32, kind="ExternalOutput")

    with tile.TileContext(nc) as tc:
        with (
            tc.tile_pool(name="const", bufs=1) as constp,
            tc.tile_pool(name="ctxT", bufs=4 * VC) as ctxTp,
            tc.tile_pool(name="ctxB", bufs=16) as ctxBp,
            tc.tile_pool(name="tmp", bufs=10) as tmpp,
            tc.tile_pool(name="sm", bufs=1) as smp,
            tc.tile_pool(name="mbp", bufs=2) as mbp,
            tc.tile_pool(name="pTp", bufs=2) as pTp,
            tc.tile_pool(name="dramp", bufs=2, space="DRAM") as dramp,
            tc.tile_pool(name="pscp", bufs=4, space="PSUM") as pscp,
            tc.tile_pool(name="pssc", bufs=1, space="PSUM") as psscp,
            tc.tile_pool(name="pswp", bufs=1, space="PSUM") as pswp,
            tc.tile_pool(name="pcmb", bufs=1, space="PSUM") as pcmbp,
        ):
            # ---- PE warmup: dummy matmuls keep the HAM activity monitor
            # busy during the ~10us NEFF/DMA startup ramp so the real
            # matmuls start at 2.4GHz instead of 1.2GHz ----
            warm_sb = constp.tile([128, SBW], BF16, tag="warm")
            nc.vector.memset(warm_sb, 0.0)
            pwarm = pcmbp.tile([128, SBW], F32, tag="cmb", name="pwarm")
            for _ in range(12):
                nc.tensor.matmul(pwarm, warm_sb[:, 0:128], warm_sb, start=True, stop=True)


            ctxT_tiles = {}  # (b, vc, half) -> tile

            def emit_ctxT_dma(b, halves=(0, 1), split=False):
                for h in halves:
                    for vc in range(VC):
                        t = ctxTp.tile(
                            [128, HW], BF16, tag="ctxT", name=f"ctxT{b}_{vc}_{h}"
                        )
                        ctxT_tiles[(b, vc, h)] = t
                    if split:
                        # two region DMAs per tile, all-vc low halves first,
                        # so the first s-block gates on 1MB instead of 2MB
                        for q in (0, 1):
                            for vc in range(VC):
                                t = ctxT_tiles[(b, vc, h)]
                                nc.sync.dma_start(
                                    out=t[:, q * SBW : (q + 1) * SBW],
                                    in_=ctxT_d[
                                        b, vc, :,
                                        h * HW + q * SBW : h * HW + (q + 1) * SBW,
                                    ],
                                )
                    else:
                        for vc in range(VC):
                            t = ctxT_tiles[(b, vc, h)]
                            nc.sync.dma_start(
                                out=t, in_=ctxT_d[b, vc, :, h * HW : (h + 1) * HW]
                            )

            # pairwise whT[vc] / ctxT(0,vc,h0) so the vc-th matmul of the
            # first accumulation group gates on only (vc+1)*0.5MB; both waits
            # are region/tile granular so the group JIT-streams the startup DMA
            whT_sb = constp.tile([128, VC, AD], BF16, tag="whT")
            for vc in range(VC):
                nc.sync.dma_start(out=whT_sb[:, vc, :], in_=whT_d[vc])
                t = ctxTp.tile([128, HW], BF16, tag="ctxT", name=f"ctxT0_{vc}_0")
                nc.sync.dma_start(out=t, in_=ctxT_d[0, vc, :, 0:HW])
                ctxT_tiles[(0, vc, 0)] = t

            # small constants on the Scalar HWDGE ring (off the big stream)
            outT_sb = constp.tile([128, QC, BL], BF16, tag="outT")
            nc.scalar.dma_start(out=outT_sb, in_=outT_d[:, :, :])
            whb_sb = constp.tile([128, AC], F32, tag="whb")
            nc.scalar.dma_start(out=whb_sb, in_=whb_d[:, :])
            vre_sb = constp.tile([128, AC], BF16, tag="vre")
            nc.scalar.dma_start(out=vre_sb, in_=vre_d[:, :])

            wsT_sb = constp.tile([128, QC, AD], BF16, tag="wsT")
            for qc in range(QC):
                nc.sync.dma_start(out=wsT_sb[:, qc, :], in_=wsT_d[qc])
            emit_ctxT_dma(0, (1,))
            emit_ctxT_dma(1)

            comb_sb = constp.tile([128, AC, BL], F32, tag="comb")

            def emit_comb():
                """comb[a,(ac,b)] = Wh_b[a] + (output @ Ws_w.T)[b,a].
                One PSUM tile, 8 accumulation regions -> no pool stalls."""
                pcmb = pcmbp.tile([128, AC * BL], F32, tag="cmb")
                for ac in range(AC):
                    reg = pcmb[:, ac * BL : (ac + 1) * BL]
                    for qc in range(QC):
                        nc.tensor.matmul(
                            reg,
                            wsT_sb[:, qc, ac * 128 : (ac + 1) * 128],
                            outT_sb[:, qc, :],
                            start=(qc == 0),
                            stop=(qc == QC - 1),
                        )
                for ac in range(AC):
                    nc.vector.tensor_scalar_add(
                        comb_sb[:, ac, :],
                        pcmb[:, ac * BL : (ac + 1) * BL],
                        whb_sb[:, ac : ac + 1],
                    )

            scores_rows = {}
            mb_tiles = {}
            pT_tiles = {}
            rs_tiles = {}
            pbf_tiles = {}
            pscr_tiles = {}
            ssum_tiles = {}

            def emit_A(b, sblocks, inject_comb=False):
                if b not in scores_rows:
                    scores_rows[b] = smp.tile(
                        [1, S], F32, tag="scores", name=f"scores{b}", bufs=2
                    )
                    mb_tiles[b] = mbp.tile([1, S], BF16, tag="mb", name=f"mb{b}")
                    nc.scalar.dma_start(out=mb_tiles[b], in_=mb_d[b])
                scores_row = scores_rows[b]
                mb_t = mb_tiles[b]

                def emit_tanh(ac, ps):
                    tm = tmpp.tile([128, SBW], BF16, tag="tm", name=f"tm{b}_{ac}")
                    nc.scalar.activation(
                        out=tm,
                        in_=ps,
                        func=AF.Tanh,
                        bias=comb_sb[:, ac, b : b + 1],
                        scale=1.0,
                    )
                    return tm

                for sb in sblocks:
                    h, ssl = sb // 2, slice((sb % 2) * SBW, (sb % 2) * SBW + SBW)
                    osl = slice(sb * SBW, (sb + 1) * SBW)
                    tmps = []
                    deferred = []
                    for ac in range(AC):
                        ps = pscp.tile([128, SBW], F32, tag="cp")
                        for vc in range(VC):
                            nc.tensor.matmul(
                                ps,
                                whT_sb[:, vc, ac * 128 : (ac + 1) * 128],
                                ctxT_tiles[(b, vc, h)][:, ssl],
                                start=(vc == 0),
                                stop=(vc == VC - 1),
                            )
                        if inject_comb and sb == sblocks[0] and ac < 4:
                            # comb_sb must be emitted before any tanh reads it
                            # (Tile tracks RAW in emission order), but the comb
                            # matmuls must also precede the 3rd accumulation
                            # group in the PE stream (pscp has 3 slots).
                            deferred.append((ac, ps))
                            tmps.append(None)
                            if ac == 3:
                                emit_comb()
                                for ac2, ps2 in deferred:
                                    tmps[ac2] = emit_tanh(ac2, ps2)
                            continue
                        tmps.append(emit_tanh(ac, ps))
                    last_sb = b == BL - 1 and sb == SB - 1
                    if last_sb:
                        # tail-latency-critical: serial accumulation avoids the
                        # 4-partial combine chain on the critical path
                        pssc = psscp.tile([128, SBW], F32, tag="sc")
                        for ac in range(AC):
                            nc.tensor.matmul(
                                pssc[0:1, :],
                                vre_sb[:, ac : ac + 1],
                                tmps[ac],
                                start=(ac == 0),
                                stop=(ac == AC - 1),
                            )
                        nc.vector.tensor_add(
                            scores_row[0:1, osl], pssc[0:1, :], mb_t[0:1, osl]
                        )
                    else:
                        pssc = psscp.tile([128, SBW], F32, tag="sc")
                        for r in range(2):
                            for j in range(4):
                                ac = r * 4 + j
                                nc.tensor.matmul(
                                    pssc[32 * j : 32 * j + 1, :],
                                    vre_sb[:, ac : ac + 1],
                                    tmps[ac],
                                    start=(r == 0),
                                    stop=(r == 1),
                                    tile_position=(0, 32 * j),
                                )
                        # combine the 4 col-group partials; mask folds into op 1
                        t0 = smp.tile([1, SBW], F32, tag="sct0", bufs=1)
                        nc.vector.tensor_add(t0, pssc[0:1, :], mb_t[0:1, osl])
                        t1 = smp.tile([1, SBW], F32, tag="sct1", bufs=1)
                        nc.vector.tensor_add(t1, pssc[32:33, :], t0)
                        t2 = smp.tile([1, SBW], F32, tag="sct2", bufs=1)
                        nc.vector.tensor_add(t2, pssc[64:65, :], t1)
                        nc.vector.tensor_add(scores_row[0:1, osl], pssc[96:97, :], t2)
                    # exp per s-block: spreads ACT work and shortens the
                    # scores->pT critical path after the last s-block
                    if b not in pbf_tiles:
                        pbf_tiles[b] = smp.tile(
                            [1, S], BF16, tag="pbf", name=f"pbf{b}", bufs=2
                        )
                        pscr_tiles[b] = dramp.tile(
                            [1, S], BF16, tag="pscr", name=f"pscr{b}"
                        )
                        ssum_tiles[b] = smp.tile(
                            [1, SB], F32, tag="ssum", name=f"ssum{b}", bufs=2
                        )
                    nc.scalar.activation(
                        out=pbf_tiles[b][0:1, osl],
                        in_=scores_row[0:1, osl],
                        func=AF.Exp,
                        accum_out=ssum_tiles[b][0:1, sb : sb + 1],
                    )
                    nc.scalar.dma_start(
                        out=pscr_tiles[b][0:1, osl], in_=pbf_tiles[b][0:1, osl]
                    )

            def emit_softmax(b):
                pbf = pbf_tiles[b]
                pT = pTp.tile([128, SCH], BF16, tag="pT", name=f"pT{b}")
                nc.scalar.dma_start(
                    out=pT,
                    in_=pscr_tiles[b].rearrange("o (r c) -> (o r) c", c=128),
                    transpose=True,
                )
                pT_tiles[b] = pT
                ssum = ssum_tiles[b]
                stot = smp.tile([1, 1], F32, tag="stot", bufs=2)
                nc.vector.reduce_sum(
                    out=stot, in_=ssum[0:1, :], axis=mybir.AxisListType.X
                )
                rs = smp.tile([1, 1], F32, tag="rs", name=f"rs{b}", bufs=2)
                nc.vector.reciprocal(out=rs, in_=stot)
                rs_tiles[b] = rs

            def emit_pout(b):
                pf = smp.tile([1, S], F32, tag="pf")
                nc.vector.tensor_scalar_mul(pf, pbf_tiles[b], rs_tiles[b][0:1, 0:1])
                nc.scalar.dma_start(out=pout_d[b], in_=pf)

            def emit_B(b):
                psw = pswp.tile([128, VD], F32, tag="w")
                pT = pT_tiles[b]
                for sc in range(SCH):
                    j, r = sc % 4, sc // 4
                    cx = ctxBp.tile([128, VD], BF16, tag="cx")
                    nc.gpsimd.dma_start(
                        out=cx, in_=ctx_d[b, sc * 128 : (sc + 1) * 128, :]
                    )
                    for vh in range(VD // SBW):
                        nc.tensor.matmul(
                            psw[32 * j : 32 * j + 1, vh * SBW : (vh + 1) * SBW],
                            pT[:, sc : sc + 1],
                            cx[:, vh * SBW : (vh + 1) * SBW],
                            start=(r == 0),
                            stop=(r == SCH // 4 - 1),
                            tile_position=(0, 32 * j),
                        )
                bt0 = smp.tile([1, VD], F32, tag="bt0")
                nc.vector.tensor_copy(out=bt0, in_=psw[0:1, :])
                bt1 = smp.tile([1, VD], F32, tag="bt1")
                nc.vector.tensor_add(bt1, psw[32:33, :], bt0)
                bt2 = smp.tile([1, VD], F32, tag="bt2")
                nc.vector.tensor_add(bt2, psw[64:65, :], bt1)
                bt3 = smp.tile([1, VD], F32, tag="bt3")
                nc.vector.tensor_add(bt3, psw[96:97, :], bt2)
                wsb = smp.tile([1, VD], F32, tag="wsb")
                # weighted = (exp(s) @ ctx) / sum(exp(s))
                nc.vector.tensor_scalar_mul(wsb, bt3, rs_tiles[b][0:1, 0:1])
                nc.scalar.dma_start(out=wout_d[b], in_=wsb)

            # ---- software-pipelined emission ----
            emit_A(0, [0], inject_comb=True)
            emit_A(0, [1, 2, 3])
            emit_ctxT_dma(2)
            emit_softmax(0)
            emit_A(1, [0])
            emit_B(0)
            emit_pout(0)
            emit_A(1, [1, 2, 3])
            emit_ctxT_dma(3)
            emit_softmax(1)
            emit_A(2, [0])
            emit_B(1)
            emit_pout(1)
            emit_A(2, [1, 2, 3])
            emit_softmax(2)
            emit_A(3, [0])
            emit_B(2)
            emit_pout(2)
            emit_A(3, [1, 2, 3])
            emit_softmax(3)
            emit_B(3)
            emit_pout(3)

    nc.compile()
    return nc


def _host_pack(output, context, attn_mask, Wh_w, Wh_b, Ws_w, v_w):
    """Build per-core in_maps with device-friendly layouts/dtypes."""
    ctx_bf = context.astype(nbf)  # [B, S, V]
    # ctxT[b, vc, p, s] = context[b, s, vc*128+p]
    ctxT_bf = np.ascontiguousarray(
        ctx_bf.transpose(0, 2, 1).reshape(B, VC, 128, S)
    )
    whT = np.ascontiguousarray(Wh_w.T.reshape(VC, 128, AD)).astype(nbf)
    wsT = np.ascontiguousarray(Ws_w.T.reshape(QC, 128, AD)).astype(nbf)
    # outT[p, qc, b] = output[b, qc*128+p]  (per core slice of b)
    outT_all = np.ascontiguousarray(
        output.T.reshape(QC, 128, B).transpose(1, 0, 2)
    ).astype(nbf)
    whb = np.ascontiguousarray(Wh_b.reshape(AC, 128).T).astype(np.float32)
    vre = np.ascontiguousarray(v_w.reshape(AC, 128).T).astype(nbf)
    mb = np.where(attn_mask[:, 0, :] == 0, np.float32(-1e9), np.float32(0.0))
    mb = np.ascontiguousarray(mb).astype(nbf)

    in_maps = []
    for c in range(N_CORES):
        bs = slice(c * BL, (c + 1) * BL)
        in_maps.append(
            {
                "ctxT": np.ascontiguousarray(ctxT_bf[bs]),
                "ctx": np.ascontiguousarray(ctx_bf[bs]),
                "whT": whT,
                "wsT": wsT,
                "outT": np.ascontiguousarray(outT_all[:, :, bs]),
                "whb": whb,
                "vre": vre,
                "mb": np.ascontiguousarray(mb[bs]),
            }
        )
    return in_maps


_NC_CACHE = []


def get_nc():
    if not _NC_CACHE:
        _NC_CACHE.append(build_nc())
    return _NC_CACHE[0]


def kernel(output, context, attn_mask, Wh_w, Wh_b, Ws_w, v_w, _trace=False):
    output = np.asarray(output, dtype=np.float32)
    context = np.asarray(context, dtype=np.float32)
    attn_mask = np.asarray(attn_mask)
    Wh_w = np.asarray(Wh_w, dtype=np.float32)
    Wh_b = np.asarray(Wh_b, dtype=np.float32)
    Ws_w = np.asarray(Ws_w, dtype=np.float32)
    v_w = np.asarray(v_w, dtype=np.float32)

    nc = get_nc()
    in_maps = _host_pack(output, context, attn_mask, Wh_w, Wh_b, Ws_w, v_w)
    kw = {}
    if _trace:
        kw = dict(trace=True)
    res = run_bass_kernel_spmd(nc, in_maps, core_ids=list(range(N_CORES)), **kw)
    weighted = np.concatenate(
        [res.results[c]["weighted"] for c in range(N_CORES)], axis=0
    ).astype(np.float32)
    p_attn = np.concatenate(
        [res.results[c]["p_attn"][:, None, :] for c in range(N_CORES)], axis=0
    ).astype(np.float32)
    if _trace:
        return (weighted, p_attn), res
    return weighted, p_attn


if __name__ == "__main__":
    rng = np.random.default_rng(0)
    inputs = {
        "output": rng.standard_normal((B, QD), dtype=np.float32),
        "context": rng.standard_normal((B, S, VD), dtype=np.float32),
        "attn_mask": rng.integers(0, 2, (B, 1, S)).astype(np.int32),
        "Wh_w": (rng.standard_normal((AD, VD), dtype=np.float32) * 0.02),
        "Wh_b": (rng.standard_normal((AD,), dtype=np.float32) * 0.02),
        "Ws_w": (rng.standard_normal((AD, QD), dtype=np.float32) * 0.02),
        "v_w": (rng.standard_normal((1, AD), dtype=np.float32) * 0.02),
    }
    w, p = kernel(**inputs)
    print("weighted", w.shape, w.dtype, "p_attn", p.shape, p.dtype)
